# revision 6
# baseline (speedup 1.0000x reference)
"""Trainium2 Bass kernel for nn_Dynamic_Q_ResMLP24 (ResMLP-24 with fake-quantized weights).

Strategy:
  - Data-parallel over batch: 64 images -> 8 cores x 8 images.
  - Host prep (numpy): replicate fq() quantization exactly, but keep the
    *integer* part of each quantized weight (exactly representable in bf16)
    and fold the scalar scales / per-channel norms / gammas / biases into
    weights and per-partition vectors algebraically.
  - Device: feature-major f32 residual stream x[d=384, t=1568] in SBUF.
    Per block: cross-patch mix via PE-transpose + bf16 matmuls, MLP
    384->1536->384 via bf16 matmuls with fused GELU on ScalarE.
    Patch-embed and head matmuls run in float32r (full PE rate, ~10-bit
    mantissa) since their error is not damped by gamma.
"""
import numpy as np
import ml_dtypes

import concourse.bass as bass
import concourse.mybir as mybir
import concourse.tile as tile
from concourse import bacc
from concourse.bass_utils import run_bass_kernel_spmd
from concourse.masks import make_identity

DIM, PATCHES, HID, NCLS, NBLK, PS = 384, 196, 1536, 1000, 24, 16
NCORES = 8
BLOC = 8                 # images per core
T = BLOC * PATCHES       # 1568 tokens per core
NCH = 4
CH = T // NCH            # 392 (= 2 images per chunk)
DT = DIM // 128          # 3 d-tiles
HT = HID // 128          # 12 h-tiles
KEMB = 768 // 128        # 6 embed K-tiles
Q = PATCHES

F32 = mybir.dt.float32
BF16 = mybir.dt.bfloat16
F32R = mybir.dt.float32r
AF = mybir.ActivationFunctionType

BF = ml_dtypes.bfloat16


def _fq_int_scale(w, bits=8):
    """fq(w) = int_part * scale, matching reference.fq in f32 arithmetic."""
    w = np.asarray(w, np.float32)
    n = np.float32(2 ** (bits - 1) - 1)
    s = np.float32(np.max(np.abs(w))) / n + np.float32(1e-8)
    q = np.clip(np.round(w / s), -n - 1.0, n).astype(np.float32)
    return q, np.float32(s)


def _host_prep(inputs):
    x = np.asarray(inputs["x"], np.float32)
    B = x.shape[0]
    assert B == NCORES * BLOC

    p = {}

    # ---- patch embed ----
    cw_int, cw_s = _fq_int_scale(inputs["conv_w"])
    xp = x.reshape(B, 3, 14, PS, 14, PS).transpose(0, 2, 4, 1, 3, 5)
    xp = np.ascontiguousarray(xp).reshape(B, PATCHES, 3 * PS * PS)
    # per-core feature-major patches [768, T]
    p["emb_x_per_core"] = [
        np.ascontiguousarray(xp[c * BLOC:(c + 1) * BLOC].reshape(T, 768).T)
        for c in range(NCORES)
    ]
    p["emb_w"] = np.ascontiguousarray(cw_int.reshape(DIM, 768).T)  # [768, 384] f32
    p["emb_scale"] = float(cw_s)
    p["conv_b"] = np.asarray(inputs["conv_b"], np.float32)

    # ---- blocks ----
    w1T = np.empty((NBLK, DIM, HID), BF)
    w2T = np.empty((NBLK, HID + 1, DIM), BF)   # row HID = g2*b2 bias row
    awT = np.empty((NBLK, Q + 1, Q), BF)       # row Q = ab bias row
    g1b = np.empty((NBLK, DIM), BF)
    vecs = np.empty((NBLK, 2, DIM), np.float32)  # afold, bfold
    b1eff = np.empty((NBLK, HID), np.float32)
    w1_scales = []
    for blk in range(NBLK):
        a1 = np.asarray(inputs["norm1_a"][blk], np.float32)
        b1 = np.asarray(inputs["norm1_b"][blk], np.float32)
        aw_int, aw_s = _fq_int_scale(inputs["attn_w"][blk])
        ab = np.asarray(inputs["attn_b"][blk], np.float32)
        g1 = np.asarray(inputs["gamma1"][blk], np.float32)
        a2 = np.asarray(inputs["norm2_a"][blk], np.float32)
        b2 = np.asarray(inputs["norm2_b"][blk], np.float32)
        w1_int, w1_s = _fq_int_scale(inputs["mlp_w1"][blk])
        bb1 = np.asarray(inputs["mlp_b1"][blk], np.float32)
        w2_int, w2_s = _fq_int_scale(inputs["mlp_w2"][blk])
        bb2 = np.asarray(inputs["mlp_b2"][blk], np.float32)
        g2 = np.asarray(inputs["gamma2"][blk], np.float32)

        # cross-patch: t1 = (g1*a1*aw_s)*x + (g1*b1*aw_s); psum = t1^T@aw_int
        # + K=1 row: g1[d] (lhsT) x ab[q] (rhs); contribution added raw.
        vecs[blk, 0] = g1 * a1 * aw_s
        vecs[blk, 1] = g1 * b1 * aw_s
        awT[blk, :Q] = aw_int.T.astype(BF)
        awT[blk, Q] = ab.astype(BF)
        g1b[blk] = g1.astype(BF)

        # MLP: fold a2 into w1 rows, b2 into b1eff; gelu(psum*w1_s + b1eff).
        # w2 folded with g2*w2_s; bias row g2*b2 via K=1 matmul with ones rhs.
        w1T[blk] = (w1_int * a2[None, :]).T.astype(BF)
        b1eff[blk] = bb1 + (w1_int * w1_s) @ b2
        w1_scales.append(float(w1_s))
        w2T[blk, :HID] = (w2_int * (w2_s * g2[:, None])).T.astype(BF)
        w2T[blk, HID] = (g2 * bb2).astype(BF)

    p["w1T"], p["w2T"], p["awT"], p["g1b"] = w1T, w2T, awT, g1b
    p["vecs"], p["b1eff"], p["w1_scales"] = vecs, b1eff, w1_scales

    # ---- head: logits = (sum_p x) * (na*hw_s/196) @ hw_int^T + hb_eff ----
    hw_int, hw_s = _fq_int_scale(inputs["head_w"])
    na = np.asarray(inputs["norm_a"], np.float32)
    nb = np.asarray(inputs["norm_b"], np.float32)
    hb = np.asarray(inputs["head_b"], np.float32)
    headw = np.empty((DIM + 1, NCLS), np.float32)
    headw[:DIM] = hw_int.T
    headw[DIM] = hb + (hw_int * hw_s) @ nb
    p["headw"] = headw
    p["nas"] = (na * hw_s / np.float32(PATCHES)).astype(np.float32)
    return p


def _build(prep, nblk=NBLK, gelu_func=None):
    if gelu_func is None:
        gelu_func = AF.Gelu
    nc = bacc.Bacc("TRN2", target_bir_lowering=False, debug=False,
                   enable_asserts=False)

    d_embx = nc.dram_tensor("emb_x", [768, T], F32R, kind="ExternalInput")
    d_embw = nc.dram_tensor("emb_w", [768, DIM], F32R, kind="ExternalInput")
    d_convb = nc.dram_tensor("conv_b", [DIM], F32, kind="ExternalInput")
    d_w1T = nc.dram_tensor("w1T", [NBLK, DIM, HID], BF16, kind="ExternalInput")
    d_w2T = nc.dram_tensor("w2T", [NBLK, HID + 1, DIM], BF16, kind="ExternalInput")
    d_awT = nc.dram_tensor("awT", [NBLK, Q + 1, Q], BF16, kind="ExternalInput")
    d_g1b = nc.dram_tensor("g1b", [NBLK, DIM], BF16, kind="ExternalInput")
    d_vecs = nc.dram_tensor("vecs", [NBLK, 2, DIM], F32, kind="ExternalInput")
    d_b1e = nc.dram_tensor("b1eff", [NBLK, HID], F32, kind="ExternalInput")
    d_headw = nc.dram_tensor("headw", [DIM + 1, NCLS], F32R, kind="ExternalInput")
    d_nas = nc.dram_tensor("nas", [DIM], F32, kind="ExternalInput")
    d_out = nc.dram_tensor("out", [BLOC, NCLS], F32, kind="ExternalOutput")

    w1s = prep["w1_scales"]
    emb_scale = prep["emb_scale"]

    with tile.TileContext(nc) as tc:
        with (
            tc.tile_pool(name="const", bufs=1) as const,
            tc.tile_pool(name="wp", bufs=2) as wp,
            tc.tile_pool(name="ap", bufs=2) as apool,
            tc.tile_pool(name="xp", bufs=1) as xpool,
            tc.tile_pool(name="gp", bufs=2) as gpool,
            tc.tile_pool(name="ep", bufs=12) as epool,
            tc.tile_pool(name="psy", bufs=2, space=bass.MemorySpace.PSUM) as ps_y,
            tc.tile_pool(name="psg", bufs=3, space=bass.MemorySpace.PSUM) as ps_g,
            tc.tile_pool(name="psm", bufs=3, space=bass.MemorySpace.PSUM) as ps_m,
        ):
            # ---- constants ----
            ident = const.tile([128, 128], BF16)
            make_identity(nc, ident)
            ones_row = const.tile([1, 512], BF16)
            nc.vector.memset(ones_row, 1.0)
            ones8f = const.tile([1, BLOC], F32)
            nc.vector.memset(ones8f, 1.0)
            ones8 = const.tile([1, BLOC], F32R)
            nc.vector.tensor_copy(out=ones8, in_=ones8f)
            convb_sb = const.tile([128, DT], F32)
            nc.sync.dma_start(out=convb_sb, in_=d_convb.ap().rearrange("(dt p) -> p dt", p=128))
            nas_sb = const.tile([128, DT], F32)
            nc.sync.dma_start(out=nas_sb, in_=d_nas.ap().rearrange("(dt p) -> p dt", p=128))
            embw_sb = const.tile([128, KEMB, DIM], F32R)
            nc.sync.dma_start(out=embw_sb, in_=d_embw.ap().rearrange("(kt p) d -> p kt d", p=128))
            headw_sb = const.tile([128, DT, NCLS], F32R)
            nc.sync.dma_start(out=headw_sb, in_=d_headw.ap()[0:DIM, :].rearrange("(kt p) n -> p kt n", p=128))
            headb_sb = const.tile([1, NCLS], F32R)
            nc.sync.dma_start(out=headb_sb, in_=d_headw.ap()[DIM:DIM + 1, :])

            x_fm = xpool.tile([128, DT, T], F32)
            sums = const.tile([128, DT, BLOC], F32)
            sums_sc = const.tile([128, DT, BLOC], F32R)
            out_sb = const.tile([BLOC, NCLS], F32)

            embx_r = d_embx.ap().rearrange("(kt p) t -> p kt t", p=128)

            def dma_weights(blk):
                w = {}
                w["w1t"] = wp.tile([128, DT, HID], BF16, tag="w1t", name=f"w1t_{blk}")
                nc.sync.dma_start(out=w["w1t"], in_=d_w1T.ap()[blk].rearrange("(kt p) h -> p kt h", p=128))
                w["w2t"] = wp.tile([128, HT, DIM], BF16, tag="w2t", name=f"w2t_{blk}")
                nc.sync.dma_start(out=w["w2t"], in_=d_w2T.ap()[blk, 0:HID, :].rearrange("(kt p) d -> p kt d", p=128))
                w["w2b"] = wp.tile([1, DIM], BF16, tag="w2b", name=f"w2b_{blk}")
                nc.sync.dma_start(out=w["w2b"], in_=d_w2T.ap()[blk, HID:HID + 1, :])
                w["awt1"] = wp.tile([128, Q], BF16, tag="awt1", name=f"awt1_{blk}")
                nc.sync.dma_start(out=w["awt1"], in_=d_awT.ap()[blk, 0:128, :])
                w["awt2"] = wp.tile([Q - 128, Q], BF16, tag="awt2", name=f"awt2_{blk}")
                nc.sync.dma_start(out=w["awt2"], in_=d_awT.ap()[blk, 128:Q, :])
                w["abg"] = wp.tile([1, Q], BF16, tag="abg", name=f"abg_{blk}")
                nc.sync.dma_start(out=w["abg"], in_=d_awT.ap()[blk, Q:Q + 1, :])
                w["g1b"] = wp.tile([1, DIM], BF16, tag="g1b", name=f"g1b_{blk}")
                nc.sync.dma_start(out=w["g1b"], in_=d_g1b.ap()[blk:blk + 1, :])
                w["vecs"] = wp.tile([128, 2, DT], F32, tag="vecs", name=f"vecs_{blk}")
                nc.sync.dma_start(out=w["vecs"], in_=d_vecs.ap()[blk].rearrange("v (dt p) -> p v dt", p=128))
                w["b1e"] = wp.tile([128, HT], F32, tag="b1e", name=f"b1e_{blk}")
                nc.sync.dma_start(out=w["b1e"], in_=d_b1e.ap()[blk].rearrange("(ht p) -> p ht", p=128))
                return w

            # ---- weights for block 0 prefetch first, then patch embed ----
            pending = dma_weights(0)

            for ch in range(NCH):
                csl = bass.ts(ch, CH)
                ext = []
                for kt in range(KEMB):
                    e = epool.tile([128, CH], F32R, tag="embx")
                    nc.sync.dma_start(out=e, in_=embx_r[:, kt, csl])
                    ext.append(e)
                for dt in range(DT):
                    pse = ps_m.tile([128, CH], F32, tag="psm")
                    for kt in range(KEMB):
                        nc.tensor.matmul(pse,
                                         embw_sb[:, kt, bass.ts(dt, 128)],
                                         ext[kt],
                                         start=(kt == 0), stop=(kt == KEMB - 1))
                    nc.scalar.activation(out=x_fm[:, dt, csl], in_=pse,
                                         func=AF.Identity,
                                         bias=convb_sb[:, dt:dt + 1],
                                         scale=emb_scale)

            # ---- blocks ----
            for blk in range(nblk):
                w = pending
                pending = dma_weights(blk + 1) if blk + 1 < nblk else None

                t1 = apool.tile([128, DT, T], BF16, tag="t1")
                t2 = apool.tile([128, DT, T], BF16, tag="t2")
                xpm1 = apool.tile([128, BLOC, DIM], BF16, tag="xpm1")
                xpm2 = apool.tile([Q - 128, BLOC, DIM], BF16, tag="xpm2")

                # phase A: cross-patch mixing
                for ch in range(NCH):
                    csl = bass.ts(ch, CH)
                    for dt in range(DT):
                        nc.vector.tensor_scalar(
                            out=t1[:, dt, csl], in0=x_fm[:, dt, csl],
                            scalar1=w["vecs"][:, 0, dt:dt + 1],
                            scalar2=w["vecs"][:, 1, dt:dt + 1],
                            op0=mybir.AluOpType.mult, op1=mybir.AluOpType.add)
                for b in range(BLOC):
                    for dt in range(DT):
                        for pt in range(2):
                            width = 128 if pt == 0 else Q - 128
                            pst = ps_m.tile([128, 128], BF16, tag="psm")
                            nc.tensor.transpose(
                                pst[0:width, :],
                                t1[:, dt, bass.ds(b * Q + pt * 128, width)],
                                ident)
                            dest = xpm1 if pt == 0 else xpm2
                            nc.vector.tensor_copy(
                                out=dest[0:width, b, bass.ts(dt, 128)],
                                in_=pst[0:width, :])
                    for dt in range(DT):
                        psy = ps_y.tile([128, Q], F32, tag="psy")
                        dsl = bass.ts(dt, 128)
                        nc.tensor.matmul(psy, xpm1[:, b, dsl], w["awt1"],
                                         start=True, stop=False)
                        nc.tensor.matmul(psy, xpm2[:, b, dsl], w["awt2"],
                                         start=False, stop=False)
                        nc.tensor.matmul(psy, w["g1b"][:, dsl], w["abg"],
                                         start=False, stop=True)
                        bsl = bass.ts(b, Q)
                        nc.vector.tensor_add(out=x_fm[:, dt, bsl],
                                             in0=x_fm[:, dt, bsl], in1=psy)

                # phase B: MLP
                for ch in range(NCH):
                    csl = bass.ts(ch, CH)
                    for dt in range(DT):
                        nc.vector.tensor_copy(out=t2[:, dt, csl], in_=x_fm[:, dt, csl])
                    g = gpool.tile([128, HT, CH], BF16, tag="g")
                    for ht in range(HT):
                        psg = ps_g.tile([128, CH], F32, tag="psg")
                        for kt in range(DT):
                            nc.tensor.matmul(psg,
                                             w["w1t"][:, kt, bass.ts(ht, 128)],
                                             t2[:, kt, csl],
                                             start=(kt == 0), stop=(kt == DT - 1))
                        nc.scalar.activation(out=g[:, ht, :], in_=psg,
                                             func=gelu_func,
                                             bias=w["b1e"][:, ht:ht + 1],
                                             scale=w1s[blk])
                    for dt in range(DT):
                        psy2 = ps_m.tile([128, CH], F32, tag="psm")
                        dsl = bass.ts(dt, 128)
                        for ht in range(HT):
                            nc.tensor.matmul(psy2, w["w2t"][:, ht, dsl],
                                             g[:, ht, :],
                                             start=(ht == 0), stop=False)
                        nc.tensor.matmul(psy2, w["w2b"][:, dsl],
                                         ones_row[:, 0:CH],
                                         start=False, stop=True)
                        nc.vector.tensor_add(out=x_fm[:, dt, csl],
                                             in0=x_fm[:, dt, csl], in1=psy2)

            # ---- head ----
            for dt in range(DT):
                nc.vector.tensor_reduce(
                    out=sums[:, dt, :],
                    in_=x_fm[:, dt, :].rearrange("p (b q) -> p b q", q=Q),
                    axis=mybir.AxisListType.X, op=mybir.AluOpType.add)
                nc.vector.tensor_scalar_mul(
                    out=sums_sc[:, dt, :], in0=sums[:, dt, :],
                    scalar1=nas_sb[:, dt:dt + 1])
            for nh in range(2):
                nsl = bass.ts(nh, NCLS // 2)
                psh = ps_m.tile([BLOC, NCLS // 2], F32, tag="psm")
                for kt in range(DT):
                    nc.tensor.matmul(psh, sums_sc[:, kt, :],
                                     headw_sb[:, kt, nsl],
                                     start=(kt == 0), stop=False)
                nc.tensor.matmul(psh, ones8,
                                 headb_sb[:, nsl],
                                 start=False, stop=True)
                nc.vector.tensor_copy(out=out_sb[:, nsl], in_=psh)
            nc.sync.dma_start(out=d_out.ap(), in_=out_sb)

    nc.compile()
    return nc


_CACHE = {}


def _get_program(prep, nblk=NBLK):
    key = ("prog", nblk, tuple(prep["w1_scales"]), prep["emb_scale"])
    if key not in _CACHE:
        _CACHE[key] = _build(prep, nblk)
    return _CACHE[key]


def make_in_maps(prep):
    shared = {
        "emb_w": prep["emb_w"], "conv_b": prep["conv_b"],
        "w1T": prep["w1T"], "w2T": prep["w2T"], "awT": prep["awT"],
        "g1b": prep["g1b"], "vecs": prep["vecs"], "b1eff": prep["b1eff"],
        "headw": prep["headw"], "nas": prep["nas"],
    }
    return [dict(shared, emb_x=prep["emb_x_per_core"][c]) for c in range(NCORES)]


def kernel(**inputs):
    prep = _host_prep(inputs)
    nc = _get_program(prep)
    in_maps = make_in_maps(prep)
    res = run_bass_kernel_spmd(nc, in_maps, core_ids=list(range(NCORES)))
    out = np.concatenate([np.asarray(res.results[c]["out"]) for c in range(NCORES)], axis=0)
    return out.astype(np.float32)


if __name__ == "__main__":
    import reference
    inputs = reference.setup_inputs()
    got = kernel(**{k: np.asarray(v) for k, v in inputs.items()})
    print("kernel out:", got.shape, got.dtype)
